# revision 16
# baseline (speedup 1.0000x reference)
"""Trainium2 Bass kernel for masked (no self-attention) attention.

reference:
    scores = (q / sqrt(d)) @ k.T          [B,H,S,S]
    scores[diag] = -1e30 ; scores[:, :, :, mask] = -1e30
    attention = softmax(scores, -1)
    sum_value = attention @ v             [B,H,S,D]
    returns (attention, sum_value)

Shapes: q/k/v [2, 8, 2048, 64] f32, mask [2, 1, 1, 2048] bool.

Sharding: 16 (b,h) pairs -> 8 NeuronCores, 2 pairs per core (pure data
parallelism, no communication).

Per-core layout strategy ("S^T primary"):
  - compute scores transposed, ST[k, q] (k on partitions), via fp32r matmuls
    lhsT = K^T block [65, 128], rhs = Q^T [65, 512].  Row 64 of K^T holds the
    additive column-mask bias and row 64 of Q^T holds ones, so the external
    mask is applied inside the same matmul (rank-1 update).  A tiny bf16
    matmul accumulates -30000 on the diagonal (no self attention).
  - exp on ScalarE (scale=1/softmax temperature folded in) -> E[k, q] in SBUF.
  - A@V: lhsT = V_aug [128, 65] (col 64 = ones), rhs = E[k, q] accumulated over
    k blocks -> sum_value^T AND the softmax row sums in one PSUM tile.
  - PE transposes E chunks back to [q, k]; VectorE multiplies by the
    reciprocal row-sum (per-partition scalar) during the PSUM->SBUF move;
    contiguous 1MB DMAs store the attention tile rows.
"""

import os
import numpy as np


def _import_concourse():
    try:
        import concourse.bass  # noqa: F401
    except ImportError:
        import sys

        for p in ("/opt/trn_rl_repo", "/root/.axon_site/_ro/trn_rl_repo"):
            if os.path.isdir(p) and p not in sys.path:
                sys.path.insert(0, p)
        import concourse.bass  # noqa: F401


B, H, S, D = 2, 8, 2048, 64
NCORES = 8
NBH = (B * H) // NCORES  # (b,h) pairs per core
P = 128
NKB = S // P  # k blocks per (b,h)
NEG = -30000.0
SCALE = float(D) ** -0.5

# Tunables (env-overridable for experiments)
E_DTYPE = os.environ.get("ATTN_E_DTYPE", "bfloat16")  # float32 | bfloat16
NQS = int(os.environ.get("ATTN_NQS", "2"))  # q splits per (b,h)
TMODE = os.environ.get("ATTN_TMODE", "0") == "1"  # transpose-mode vs matmul-by-identity
E_BUFS = int(os.environ.get("ATTN_E_BUFS", "0"))  # 0 = auto
PHASES = int(os.environ.get("ATTN_PHASES", "3"))  # 1=QK+exp 2=+AV/sv 3=full


def _build_nc():
    _import_concourse()
    import concourse.bass as bass
    import concourse.mybir as mybir
    import concourse.tile as tile
    from concourse import bacc
    from contextlib import ExitStack

    f32 = mybir.dt.float32
    f32r = mybir.dt.float32r
    bf16 = mybir.dt.bfloat16
    e_dt = f32 if E_DTYPE == "float32" else bf16
    # dtype used for SBUF tiles that feed the big matmuls: walrus requires
    # operands of fp32r matmuls to be *produced* as float32r
    op_dt = f32r if e_dt == f32 else bf16
    ds = bass.ds

    QH = S // NQS  # q rows per phase
    NQB = QH // P  # q blocks per phase
    NQC = QH // 512  # 512-wide q chunks per phase
    e_bufs = E_BUFS or ((2 * NKB + 2) if e_dt == bf16 else (NKB + 6))

    nc = bacc.Bacc("TRN2", target_bir_lowering=False, debug=False, num_devices=NCORES)

    qd = nc.dram_tensor("q", [NBH, S, D], f32, kind="ExternalInput").ap()
    kd = nc.dram_tensor("k", [NBH, S, D], f32, kind="ExternalInput").ap()
    vd = nc.dram_tensor("v", [NBH, S, D], f32, kind="ExternalInput").ap()
    mbias = nc.dram_tensor("mbias", [NBH, S], f32, kind="ExternalInput").ap()
    attn_d = nc.dram_tensor("attn", [NBH, S, S], f32, kind="ExternalOutput").ap()
    sv_d = nc.dram_tensor("sv", [NBH, S, D], f32, kind="ExternalOutput").ap()

    EXP = mybir.ActivationFunctionType.Exp

    with tile.TileContext(nc) as tc:
        with ExitStack() as ctx:
            const = ctx.enter_context(tc.tile_pool(name="const", bufs=1))
            ktq = ctx.enter_context(tc.tile_pool(name="ktq", bufs=2))
            nat = ctx.enter_context(tc.tile_pool(name="nat", bufs=3))
            vpool = ctx.enter_context(tc.tile_pool(name="vp", bufs=2))
            epool = ctx.enter_context(tc.tile_pool(name="ep", bufs=e_bufs))
            apool = ctx.enter_context(tc.tile_pool(name="aout", bufs=3))
            svsb = ctx.enter_context(tc.tile_pool(name="svsb", bufs=2))
            svout = ctx.enter_context(tc.tile_pool(name="svout", bufs=2))
            recp = ctx.enter_context(tc.tile_pool(name="recp", bufs=3))
            pst = ctx.enter_context(tc.tile_pool(name="pst", bufs=2, space="PSUM"))
            pat = ctx.enter_context(tc.tile_pool(name="pat", bufs=3, space="PSUM"))
            psv = ctx.enter_context(tc.tile_pool(name="psv", bufs=1, space="PSUM"))

            # constants: identities + diagonal(NEG) tile
            ident_f32 = const.tile([P, P], f32, tag="idf32")
            nc.gpsimd.memset(ident_f32, 0.0)
            nc.gpsimd.affine_select(
                out=ident_f32, in_=ident_f32,
                compare_op=mybir.AluOpType.not_equal,
                fill=1.0, base=0, pattern=[[-1, P]], channel_multiplier=1,
            )
            ident_bf = const.tile([P, P], bf16, tag="idbf")
            nc.gpsimd.memset(ident_bf, 0.0)
            nc.gpsimd.affine_select(
                out=ident_bf, in_=ident_bf,
                compare_op=mybir.AluOpType.not_equal,
                fill=1.0, base=0, pattern=[[-1, P]], channel_multiplier=1,
            )
            if e_dt == f32:
                ident_e = const.tile([P, P], f32r, tag="idf32r")
                nc.scalar.copy(ident_e, ident_f32)
            else:
                ident_e = ident_bf
            diagneg = const.tile([P, P], bf16, tag="diagneg")
            nc.gpsimd.memset(diagneg, 0.0)
            nc.gpsimd.affine_select(
                out=diagneg, in_=diagneg,
                compare_op=mybir.AluOpType.not_equal,
                fill=NEG, base=0, pattern=[[-1, P]], channel_multiplier=1,
            )

            qts, kts, vas = {}, {}, {}

            def input_stage(ibh):
                # load Q, K; transpose into augmented [65, S] f32r tiles.
                # All writes into qt/kt/v_aug go through ScalarE so consumers
                # wait on a single engine clock.
                qt = ktq.tile([D + 1, S], op_dt, tag="qt")
                kt = ktq.tile([D + 1, S], op_dt, tag="kt")
                qts[ibh], kts[ibh] = qt, kt
                for g, src, dstt in (
                    [(0, qd, qt), (1, qd, qt), (0, kd, kt), (1, kd, kt),
                     (2, kd, kt), (3, kd, kt), (2, qd, qt), (3, qd, qt)]
                ):
                    if True:
                        stg = nat.tile([P, 4, D], f32, tag="nat")
                        nc.sync.dma_start(
                            out=stg,
                            in_=src[ibh][ds(g * 512, 512), :].rearrange(
                                "(t p) d -> p t d", p=P
                            ),
                        )
                        ps = pst.tile([D, 512], f32, tag="ps")
                        for t in range(4):
                            nc.tensor.matmul(
                                ps[:, ds(t * P, P)], stg[:, t, :], ident_f32,
                                start=True, stop=True,
                            )
                        nc.scalar.copy(dstt[0:D, ds(g * 512, 512)], ps)
                mstg = nat.tile([D + 1, S], f32, tag="mstg")
                nc.sync.dma_start(out=mstg[D : D + 1, :], in_=mbias[ds(ibh, 1), :])
                nc.scalar.activation(
                    qt[D : D + 1, :], mstg[D : D + 1, :],
                    func=mybir.ActivationFunctionType.Copy,
                    scale=0.0, bias=1.0,
                )
                nc.scalar.copy(kt[D : D + 1, :], mstg[D : D + 1, :])

                v_aug = vpool.tile([P, NKB, D + 1], op_dt, tag="v")
                vas[ibh] = v_aug
                vstg = nat.tile([P, NKB, D], f32, tag="vstg")
                nc.sync.dma_start(
                    out=vstg, in_=vd[ibh].rearrange("(t p) d -> p t d", p=P)
                )
                nc.scalar.copy(v_aug[:, :, 0:D], vstg)
                nc.scalar.activation(
                    v_aug[:, :, D : D + 1], v_aug[:, :, 0:1],
                    func=mybir.ActivationFunctionType.Copy,
                    scale=0.0, bias=1.0,
                )

            def qk_exp(ibh, ih, prev=None):
                # scores^T (+mask bias via aug row, +diag via bf16 matmul),
                # one exp per k-block -> E[k, q].  The previous unit's output
                # blocks are interleaved between k blocks to keep PE dense.
                q0 = ih * QH
                qt, kt = qts[ibh], kts[ibh]
                etiles = []
                for kb in range(NKB):
                    ekb = epool.tile([P, QH], op_dt, tag="e")
                    etiles.append(ekb)
                    st = pst.tile([P, QH], f32, tag="ps")
                    for c in range(NQC):
                        qlo = q0 + c * 512
                        has_diag = qlo <= kb * P < qlo + 512
                        nc.tensor.matmul(
                            st[:, ds(c * 512, 512)],
                            kt[:, ds(kb * P, P)],
                            qt[:, ds(qlo, 512)],
                            start=True,
                            stop=not has_diag,
                        )
                        if has_diag:
                            off = kb * P - qlo
                            nc.tensor.matmul(
                                st[:, ds(off + c * 0, P) if False else ds(kb * P - q0, P)],
                                diagneg,
                                ident_bf,
                                start=False,
                                stop=True,
                            )
                    nc.scalar.activation(ekb, st, func=EXP, scale=SCALE)
                return etiles

            def av_sums(ibh, ih, etiles, qc_hook=None):
                # sum_value^T plus softmax row sums in one PSUM accumulation.
                # Reciprocals are produced per q-chunk so dependents can start
                # before the whole unit's A@V finishes.
                q0 = ih * QH
                v_aug = vas[ibh]
                JPC = 512 // P  # q blocks per chunk
                svT = svsb.tile([D + 1, QH], f32, tag="svT")
                rec = recp.tile([P, NQB], f32, tag="rec")
                svo = svout.tile([P, NQB, D], f32, tag="svo")
                for qc in range(NQC):
                    svp = psv.tile([D + 1, 512], f32, tag="sv")
                    for kb in range(NKB):
                        nc.tensor.matmul(
                            svp,
                            v_aug[:, kb, :],
                            etiles[kb][:, ds(qc * 512, 512)],
                            start=(kb == 0),
                            stop=(kb == NKB - 1),
                        )
                    nc.vector.tensor_copy(svT[:, ds(qc * 512, 512)], svp)
                    sums_t = pst.tile([P, JPC], f32, tag="ps")
                    for jj in range(JPC):
                        j = qc * JPC + jj
                        nc.tensor.matmul(
                            sums_t[:, ds(jj, 1)],
                            svT[D : D + 1, ds(j * P, P)],
                            ident_f32[D : D + 1, D : D + 1],
                            start=True, stop=True,
                        )
                    nc.vector.reciprocal(rec[:, ds(qc * JPC, JPC)], sums_t)
                    for jj in range(JPC):
                        j = qc * JPC + jj
                        ps2 = pst.tile([P, D], f32, tag="ps")
                        nc.tensor.matmul(
                            ps2, svT[0:D, ds(j * P, P)], ident_f32[0:D, 0:D],
                            start=True, stop=True,
                        )
                        nc.vector.tensor_scalar_mul(
                            svo[:, j, :], ps2, rec[:, ds(j, 1)]
                        )
                    if qc_hook is not None:
                        qc_hook(qc, rec)
                nc.sync.dma_start(
                    out=sv_d[ibh][ds(q0, QH), :].rearrange("(j p) d -> p j d", p=P),
                    in_=svo,
                )
                return rec

            def out_jq(ibh, ih, etiles, rec, jq):
                # transpose E -> A[q, k] for one q block, normalize, store
                q0 = ih * QH
                a_sb = apool.tile([P, S], f32, tag="a")
                for kc in range(4):
                    at = pat.tile([P, 512], op_dt if TMODE else f32, tag="at")
                    for t in range(4):
                        esl = etiles[kc * 4 + t][:, ds(jq * P, P)]
                        if TMODE:
                            nc.tensor.transpose(at[:, ds(t * P, P)], esl, ident_e)
                        else:
                            nc.tensor.matmul(
                                at[:, ds(t * P, P)], esl, ident_e,
                                start=True, stop=True,
                            )
                    at_read = at.bitcast(f32) if at.dtype == f32r else at
                    nc.vector.tensor_scalar_mul(
                        a_sb[:, ds(kc * 512, 512)], at_read,
                        rec[:, ds(jq, 1)],
                    )
                nc.sync.dma_start(
                    out=attn_d[ibh][ds(q0 + jq * P, P), :], in_=a_sb
                )

            # software pipeline: emit unit i's output stage after unit i+1's
            # compute stages so the DVE-heavy normalize always overlaps dense
            # PE work on the next unit
            units = [(ibh, ih) for ibh in range(NBH) for ih in range(NQS)]
            JPC = 512 // P
            pending = None
            for idx, (ibh, ih) in enumerate(units):
                if ih == 0:
                    input_stage(ibh)
                etiles = qk_exp(ibh, ih)
                last = idx == len(units) - 1

                def qc_hook(qc, rec, ibh=ibh, ih=ih, etiles=etiles):
                    for jj in range(JPC):
                        out_jq(ibh, ih, etiles, rec, qc * JPC + jj)

                rec = av_sums(
                    ibh, ih, etiles, qc_hook=qc_hook if last else None
                )
                if pending is not None:
                    for jq in range(NQB):
                        out_jq(*pending, jq)
                pending = None if last else (ibh, ih, etiles, rec)

    nc.compile()
    return nc


_NC_CACHE = None


def _get_nc():
    global _NC_CACHE
    if _NC_CACHE is None:
        _NC_CACHE = _build_nc()
    return _NC_CACHE


def make_in_maps(query, key, value, mask):
    """Shard full inputs into per-core input maps (2 (b,h) pairs per core)."""
    q = np.ascontiguousarray(query, np.float32).reshape(B * H, S, D)
    k = np.ascontiguousarray(key, np.float32).reshape(B * H, S, D)
    v = np.ascontiguousarray(value, np.float32).reshape(B * H, S, D)
    mb = np.where(np.asarray(mask).reshape(B, S), np.float32(NEG), np.float32(0.0))
    mb = mb.astype(np.float32)
    in_maps = []
    for c in range(NCORES):
        pairs = [c * NBH + i for i in range(NBH)]
        in_maps.append(
            {
                "q": np.ascontiguousarray(q[pairs]),
                "k": np.ascontiguousarray(k[pairs]),
                "v": np.ascontiguousarray(v[pairs]),
                "mbias": np.ascontiguousarray(
                    np.stack([mb[p // H] for p in pairs])
                ),
            }
        )
    return in_maps


def run_sharded(query, key, value, mask, trace=False):
    """Run on all 8 cores; returns (attention, sum_value, BassKernelResults)."""
    _import_concourse()
    from concourse.bass_utils import run_bass_kernel_spmd

    nc = _get_nc()
    in_maps = make_in_maps(query, key, value, mask)
    br = run_bass_kernel_spmd(nc, in_maps, list(range(NCORES)), trace=trace)
    attn = np.concatenate([r["attn"] for r in br.results]).reshape(B, H, S, S)
    sv = np.concatenate([r["sv"] for r in br.results]).reshape(B, H, S, D)
    return attn, sv, br


def kernel(query, key, value, mask):
    attn, sv, _ = run_sharded(query, key, value, mask)
    return attn, sv


# revision 17
# speedup vs baseline: 1.2038x; 1.2038x over previous
"""Trainium2 Bass kernel for masked (no self-attention) attention.

reference:
    scores = (q / sqrt(d)) @ k.T          [B,H,S,S]
    scores[diag] = -1e30 ; scores[:, :, :, mask] = -1e30
    attention = softmax(scores, -1)
    sum_value = attention @ v             [B,H,S,D]
    returns (attention, sum_value)

Shapes: q/k/v [2, 8, 2048, 64] f32, mask [2, 1, 1, 2048] bool.

Sharding: 16 (b,h) pairs -> 8 NeuronCores, 2 pairs per core (pure data
parallelism, no communication).

Per-core layout strategy ("S^T primary"):
  - compute scores transposed, ST[k, q] (k on partitions), via fp32r matmuls
    lhsT = K^T block [65, 128], rhs = Q^T [65, 512].  Row 64 of K^T holds the
    additive column-mask bias and row 64 of Q^T holds ones, so the external
    mask is applied inside the same matmul (rank-1 update).  A tiny bf16
    matmul accumulates -30000 on the diagonal (no self attention).
  - exp on ScalarE (scale=1/softmax temperature folded in) -> E[k, q] in SBUF.
  - A@V: lhsT = V_aug [128, 65] (col 64 = ones), rhs = E[k, q] accumulated over
    k blocks -> sum_value^T AND the softmax row sums in one PSUM tile.
  - PE transposes E chunks back to [q, k]; VectorE multiplies by the
    reciprocal row-sum (per-partition scalar) during the PSUM->SBUF move;
    contiguous 1MB DMAs store the attention tile rows.
"""

import os
import numpy as np


def _import_concourse():
    try:
        import concourse.bass  # noqa: F401
    except ImportError:
        import sys

        for p in ("/opt/trn_rl_repo", "/root/.axon_site/_ro/trn_rl_repo"):
            if os.path.isdir(p) and p not in sys.path:
                sys.path.insert(0, p)
        import concourse.bass  # noqa: F401


B, H, S, D = 2, 8, 2048, 64
NCORES = 8
NBH = (B * H) // NCORES  # (b,h) pairs per core
P = 128
NKB = S // P  # k blocks per (b,h)
NEG = -30000.0
SCALE = float(D) ** -0.5

# Tunables (env-overridable for experiments)
E_DTYPE = os.environ.get("ATTN_E_DTYPE", "bfloat16")  # float32 | bfloat16
NQS = int(os.environ.get("ATTN_NQS", "2"))  # q splits per (b,h)
TMODE = os.environ.get("ATTN_TMODE", "0") == "1"  # transpose-mode vs matmul-by-identity
E_BUFS = int(os.environ.get("ATTN_E_BUFS", "0"))  # 0 = auto
PHASES = int(os.environ.get("ATTN_PHASES", "3"))  # 1=QK+exp 2=+AV/sv 3=full


def _build_nc():
    _import_concourse()
    import concourse.bass as bass
    import concourse.mybir as mybir
    import concourse.tile as tile
    from concourse import bacc
    from contextlib import ExitStack

    f32 = mybir.dt.float32
    f32r = mybir.dt.float32r
    bf16 = mybir.dt.bfloat16
    e_dt = f32 if E_DTYPE == "float32" else bf16
    # dtype used for SBUF tiles that feed the big matmuls: walrus requires
    # operands of fp32r matmuls to be *produced* as float32r
    op_dt = f32r if e_dt == f32 else bf16
    ds = bass.ds

    QH = S // NQS  # q rows per phase
    NQB = QH // P  # q blocks per phase
    NQC = QH // 512  # 512-wide q chunks per phase
    e_bufs = E_BUFS or ((2 * NKB + 2) if e_dt == bf16 else (NKB + 6))

    nc = bacc.Bacc("TRN2", target_bir_lowering=False, debug=False, num_devices=NCORES)

    qd = nc.dram_tensor("q", [NBH, S, D], f32, kind="ExternalInput").ap()
    kd = nc.dram_tensor("k", [NBH, S, D], f32, kind="ExternalInput").ap()
    vd = nc.dram_tensor("v", [NBH, S, D], f32, kind="ExternalInput").ap()
    mbias = nc.dram_tensor("mbias", [NBH, S], f32, kind="ExternalInput").ap()
    attn_d = nc.dram_tensor("attn", [NBH, S, S], f32, kind="ExternalOutput").ap()
    sv_d = nc.dram_tensor("sv", [NBH, S, D], f32, kind="ExternalOutput").ap()

    EXP = mybir.ActivationFunctionType.Exp

    with tile.TileContext(nc) as tc:
        with ExitStack() as ctx:
            const = ctx.enter_context(tc.tile_pool(name="const", bufs=1))
            ktq = ctx.enter_context(tc.tile_pool(name="ktq", bufs=2))
            nat = ctx.enter_context(tc.tile_pool(name="nat", bufs=3))
            vpool = ctx.enter_context(tc.tile_pool(name="vp", bufs=2))
            epool = ctx.enter_context(tc.tile_pool(name="ep", bufs=e_bufs))
            apool = ctx.enter_context(tc.tile_pool(name="aout", bufs=3))
            svsb = ctx.enter_context(tc.tile_pool(name="svsb", bufs=2))
            svout = ctx.enter_context(tc.tile_pool(name="svout", bufs=2))
            recp = ctx.enter_context(tc.tile_pool(name="recp", bufs=3))
            pst = ctx.enter_context(tc.tile_pool(name="pst", bufs=2, space="PSUM"))
            pat = ctx.enter_context(tc.tile_pool(name="pat", bufs=3, space="PSUM"))
            psv = ctx.enter_context(tc.tile_pool(name="psv", bufs=1, space="PSUM"))

            # constants: identities + diagonal(NEG) tile
            ident_f32 = const.tile([P, P], f32, tag="idf32")
            nc.gpsimd.memset(ident_f32, 0.0)
            nc.gpsimd.affine_select(
                out=ident_f32, in_=ident_f32,
                compare_op=mybir.AluOpType.not_equal,
                fill=1.0, base=0, pattern=[[-1, P]], channel_multiplier=1,
            )
            ident_bf = const.tile([P, P], bf16, tag="idbf")
            nc.gpsimd.memset(ident_bf, 0.0)
            nc.gpsimd.affine_select(
                out=ident_bf, in_=ident_bf,
                compare_op=mybir.AluOpType.not_equal,
                fill=1.0, base=0, pattern=[[-1, P]], channel_multiplier=1,
            )
            if e_dt == f32:
                ident_e = const.tile([P, P], f32r, tag="idf32r")
                nc.scalar.copy(ident_e, ident_f32)
            else:
                ident_e = ident_bf
            diagneg = const.tile([P, P], bf16, tag="diagneg")
            nc.gpsimd.memset(diagneg, 0.0)
            nc.gpsimd.affine_select(
                out=diagneg, in_=diagneg,
                compare_op=mybir.AluOpType.not_equal,
                fill=NEG, base=0, pattern=[[-1, P]], channel_multiplier=1,
            )

            qts, kts, vas = {}, {}, {}

            def input_stage(ibh):
                # load Q, K; transpose into augmented [65, S] f32r tiles.
                # All writes into qt/kt/v_aug go through ScalarE so consumers
                # wait on a single engine clock.
                qt = ktq.tile([D + 1, S], op_dt, tag="qt")
                kt = ktq.tile([D + 1, S], op_dt, tag="kt")
                qts[ibh], kts[ibh] = qt, kt
                for g, src, dstt in (
                    [(0, qd, qt), (1, qd, qt), (0, kd, kt), (1, kd, kt),
                     (2, kd, kt), (3, kd, kt), (2, qd, qt), (3, qd, qt)]
                ):
                    if True:
                        stg = nat.tile([P, 4, D], f32, tag="nat")
                        nc.sync.dma_start(
                            out=stg,
                            in_=src[ibh][ds(g * 512, 512), :].rearrange(
                                "(t p) d -> p t d", p=P
                            ),
                        )
                        ps = pst.tile([D, 512], f32, tag="ps")
                        for t in range(4):
                            nc.tensor.matmul(
                                ps[:, ds(t * P, P)], stg[:, t, :], ident_f32,
                                start=True, stop=True,
                            )
                        nc.scalar.copy(dstt[0:D, ds(g * 512, 512)], ps)
                mstg = nat.tile([D + 1, S], f32, tag="mstg")
                nc.sync.dma_start(out=mstg[D : D + 1, :], in_=mbias[ds(ibh, 1), :])
                nc.scalar.activation(
                    qt[D : D + 1, :], mstg[D : D + 1, :],
                    func=mybir.ActivationFunctionType.Copy,
                    scale=0.0, bias=1.0,
                )
                nc.scalar.copy(kt[D : D + 1, :], mstg[D : D + 1, :])

                v_aug = vpool.tile([P, NKB, D + 1], op_dt, tag="v")
                vas[ibh] = v_aug
                vstg = nat.tile([P, NKB, D], f32, tag="vstg")
                nc.sync.dma_start(
                    out=vstg, in_=vd[ibh].rearrange("(t p) d -> p t d", p=P)
                )
                nc.scalar.copy(v_aug[:, :, 0:D], vstg)
                nc.scalar.activation(
                    v_aug[:, :, D : D + 1], v_aug[:, :, 0:1],
                    func=mybir.ActivationFunctionType.Copy,
                    scale=0.0, bias=1.0,
                )

            def qk_exp(ibh, ih, prev=None):
                # scores^T (+mask bias via aug row, +diag via bf16 matmul),
                # one exp per k-block -> E[k, q].  The previous unit's output
                # blocks are interleaved between k blocks to keep PE dense.
                q0 = ih * QH
                qt, kt = qts[ibh], kts[ibh]
                etiles = []
                for kb in range(NKB):
                    ekb = epool.tile([P, QH], op_dt, tag="e")
                    etiles.append(ekb)
                    st = pst.tile([P, QH], f32, tag="ps")
                    for c in range(NQC):
                        qlo = q0 + c * 512
                        has_diag = qlo <= kb * P < qlo + 512
                        nc.tensor.matmul(
                            st[:, ds(c * 512, 512)],
                            kt[:, ds(kb * P, P)],
                            qt[:, ds(qlo, 512)],
                            start=True,
                            stop=not has_diag,
                        )
                        if has_diag:
                            off = kb * P - qlo
                            nc.tensor.matmul(
                                st[:, ds(off + c * 0, P) if False else ds(kb * P - q0, P)],
                                diagneg,
                                ident_bf,
                                start=False,
                                stop=True,
                            )
                    nc.scalar.activation(ekb, st, func=EXP, scale=SCALE)
                return etiles

            def av_sums(ibh, ih, etiles):
                # sum_value^T plus softmax row sums in one PSUM accumulation.
                # Reciprocals are produced per q-chunk so dependents can start
                # before the whole unit's A@V finishes.
                q0 = ih * QH
                v_aug = vas[ibh]
                JPC = 512 // P  # q blocks per chunk
                svT = svsb.tile([D + 1, QH], f32, tag="svT")
                rec = recp.tile([P, NQB], f32, tag="rec")
                svo = svout.tile([P, NQB, D], f32, tag="svo")
                for qc in range(NQC):
                    svp = psv.tile([D + 1, 512], f32, tag="sv")
                    for kb in range(NKB):
                        nc.tensor.matmul(
                            svp,
                            v_aug[:, kb, :],
                            etiles[kb][:, ds(qc * 512, 512)],
                            start=(kb == 0),
                            stop=(kb == NKB - 1),
                        )
                    nc.vector.tensor_copy(svT[:, ds(qc * 512, 512)], svp)
                    sums_t = pst.tile([P, JPC], f32, tag="ps")
                    for jj in range(JPC):
                        j = qc * JPC + jj
                        nc.tensor.matmul(
                            sums_t[:, ds(jj, 1)],
                            svT[D : D + 1, ds(j * P, P)],
                            ident_f32[D : D + 1, D : D + 1],
                            start=True, stop=True,
                        )
                    nc.vector.reciprocal(rec[:, ds(qc * JPC, JPC)], sums_t)
                    for jj in range(JPC):
                        j = qc * JPC + jj
                        ps2 = pst.tile([P, D], f32, tag="ps")
                        nc.tensor.matmul(
                            ps2, svT[0:D, ds(j * P, P)], ident_f32[0:D, 0:D],
                            start=True, stop=True,
                        )
                        nc.vector.tensor_scalar_mul(
                            svo[:, j, :], ps2, rec[:, ds(j, 1)]
                        )
                nc.sync.dma_start(
                    out=sv_d[ibh][ds(q0, QH), :].rearrange("(j p) d -> p j d", p=P),
                    in_=svo,
                )
                return rec

            def out_jq(ibh, ih, etiles, rec, jq):
                # transpose E -> A[q, k] for one q block, normalize, store
                q0 = ih * QH
                a_sb = apool.tile([P, S], f32, tag="a")
                for kc in range(4):
                    at = pat.tile([P, 512], op_dt if TMODE else f32, tag="at")
                    for t in range(4):
                        esl = etiles[kc * 4 + t][:, ds(jq * P, P)]
                        if TMODE:
                            nc.tensor.transpose(at[:, ds(t * P, P)], esl, ident_e)
                        else:
                            nc.tensor.matmul(
                                at[:, ds(t * P, P)], esl, ident_e,
                                start=True, stop=True,
                            )
                    at_read = at.bitcast(f32) if at.dtype == f32r else at
                    nc.vector.tensor_scalar_mul(
                        a_sb[:, ds(kc * 512, 512)], at_read,
                        rec[:, ds(jq, 1)],
                    )
                nc.sync.dma_start(
                    out=attn_d[ibh][ds(q0 + jq * P, P), :], in_=a_sb
                )

            # software pipeline: emit unit i's output stage after unit i+1's
            # compute stages so the DVE-heavy normalize always overlaps dense
            # PE work on the next unit
            units = [(ibh, ih) for ibh in range(NBH) for ih in range(NQS)]
            pending = None
            for ibh, ih in units:
                if ih == 0:
                    input_stage(ibh)
                etiles = qk_exp(ibh, ih)
                rec = av_sums(ibh, ih, etiles)
                if pending is not None:
                    for jq in range(NQB):
                        out_jq(*pending, jq)
                pending = (ibh, ih, etiles, rec)
            for jq in range(NQB):
                out_jq(*pending, jq)

    nc.compile()
    return nc


_NC_CACHE = None


def _get_nc():
    global _NC_CACHE
    if _NC_CACHE is None:
        _NC_CACHE = _build_nc()
    return _NC_CACHE


def make_in_maps(query, key, value, mask):
    """Shard full inputs into per-core input maps (2 (b,h) pairs per core)."""
    q = np.ascontiguousarray(query, np.float32).reshape(B * H, S, D)
    k = np.ascontiguousarray(key, np.float32).reshape(B * H, S, D)
    v = np.ascontiguousarray(value, np.float32).reshape(B * H, S, D)
    mb = np.where(np.asarray(mask).reshape(B, S), np.float32(NEG), np.float32(0.0))
    mb = mb.astype(np.float32)
    in_maps = []
    for c in range(NCORES):
        pairs = [c * NBH + i for i in range(NBH)]
        in_maps.append(
            {
                "q": np.ascontiguousarray(q[pairs]),
                "k": np.ascontiguousarray(k[pairs]),
                "v": np.ascontiguousarray(v[pairs]),
                "mbias": np.ascontiguousarray(
                    np.stack([mb[p // H] for p in pairs])
                ),
            }
        )
    return in_maps


def run_sharded(query, key, value, mask, trace=False):
    """Run on all 8 cores; returns (attention, sum_value, BassKernelResults)."""
    _import_concourse()
    from concourse.bass_utils import run_bass_kernel_spmd

    nc = _get_nc()
    in_maps = make_in_maps(query, key, value, mask)
    br = run_bass_kernel_spmd(nc, in_maps, list(range(NCORES)), trace=trace)
    attn = np.concatenate([r["attn"] for r in br.results]).reshape(B, H, S, S)
    sv = np.concatenate([r["sv"] for r in br.results]).reshape(B, H, S, D)
    return attn, sv, br


def kernel(query, key, value, mask):
    attn, sv, _ = run_sharded(query, key, value, mask)
    return attn, sv
